# revision 17
# baseline (speedup 1.0000x reference)
"""Bezier stroke renderer on 8 Trainium2 NeuronCores (Bass/Tile SPMD kernel).

Reference semantics: 32 cubic-Bezier strokes, each sampled into a 16-segment
polyline, rasterized onto a 1024x1024 canvas: per pixel and segment,
darkness = clip((2t - dist_to_segment)/(2t), 0, 1), max over segments within a
stroke, then grid = max(grid, darkness * color) over strokes (3 channels).

Strategy (sharding: spatial split of the pixel grid by rows):
  - The canvas is split into 16 blocks of 64 rows; each core owns 2 blocks
    (greedy-balanced by estimated work), giving a [128 partitions x 1024 cols]
    canvas tile per core held in SBUF.
  - Only pixels within 2t+1 of a segment can be painted, so host code builds a
    worklist of (segment, block) windows, chunked into fixed 32-column items.
    All per-item parameters are shipped as per-core data tables; the single
    SPMD instruction stream is identical across cores (counts padded to the
    max over cores).
  - Distance math in the segment's tangent/normal frame, pre-scaled by 1/(2t):
        dist/(2t) = sqrt(relu(a-L)^2 + min(a,0)^2 + b^2)
    where a,b are affine in pixel coords -> computed by TensorE matmuls
    (lhsT = [x_p; 1], per-column coefficients from host tables).
  - Per channel, w_c = (dist/(2t) - 1) * col_c is min-composited into a
    negated-grid accumulator via register-offset dynamic windows (scatter),
    split across the DVE and GPSIMD engines with separate accumulators.
  - Final: out = -min(grid_dve, grid_gps), DMA to DRAM.
"""

import sys
import types
import contextlib
import ctypes

sys.path.insert(0, "/opt/trn_rl_repo")

import numpy as np

G = 1024
P = 16
N = 32
N_CORES = 8
BH = 64           # block height (rows)
NB = G // BH      # 16 blocks
BLOCKS_PER_CORE = NB // N_CORES
W_ITEM = 32       # columns per packed chunk-item
MAX_CLASS = 2     # scatter windows are 1..MAX_CLASS chunk-items wide
CHUNK = 512       # packed columns per matmul/PSUM chunk
ITEMS_PER_CHUNK = CHUNK // W_ITEM  # 16

_PROG_CACHE = {}
_HOOK_INSTALLED = False


def _install_ntff_hook():
    """Register the NTFF profile hook (mirrors trn_boot.py) so
    run_bass_kernel_spmd(trace=True) can measure HW exec time."""
    global _HOOK_INSTALLED
    if _HOOK_INSTALLED:
        return
    _HOOK_INSTALLED = True
    try:
        import antenv
        mod = types.ModuleType("antenv.axon_hooks")
        holder = [None]
        mod.set_axon_ntff_profile_hook = lambda h: holder.__setitem__(0, h)
        mod.get_axon_ntff_profile_hook = lambda: holder[0]
        sys.modules["antenv.axon_hooks"] = mod
        antenv.axon_hooks = mod

        lib = ctypes.CDLL("/opt/axon/libaxon_pjrt.so")
        if not hasattr(lib, "axon_start_nrt_profile"):
            return
        lib.axon_start_nrt_profile.argtypes = [
            ctypes.POINTER(ctypes.c_int64),
            ctypes.c_size_t,
        ]
        lib.axon_start_nrt_profile.restype = ctypes.c_int64
        lib.axon_stop_nrt_profile.argtypes = [ctypes.c_char_p]
        lib.axon_stop_nrt_profile.restype = ctypes.c_int64

        @contextlib.contextmanager
        def _hook(output_dir, device_ids):
            import jax
            jax.devices()
            if device_ids:
                ids = (ctypes.c_int64 * len(device_ids))(*device_ids)
                rc = lib.axon_start_nrt_profile(ids, len(device_ids))
            else:
                rc = lib.axon_start_nrt_profile(None, 0)
            if rc != 0:
                raise RuntimeError(f"axon_start_nrt_profile rc={rc}")
            try:
                yield
            finally:
                n = lib.axon_stop_nrt_profile(str(output_dir).encode())
                print(f"profile: {n} file(s) written to {output_dir}",
                      file=sys.stderr)

        mod.set_axon_ntff_profile_hook(_hook)
    except Exception:
        pass


# ---------------------------------------------------------------- host side

def _bezier_weights_f32(p):
    t = np.arange(p, dtype=np.float64)
    w1 = (p - t) ** 3 / p ** 3
    w2 = 3 * (p - t) ** 2 * t / p ** 3
    w3 = 3 * (p - t) * t ** 2 / p ** 3
    w4 = t ** 3 / p ** 3
    return np.stack([w1, w2, w3, w4]).astype(np.float32)  # (4, P)


def _polylines(strokes):
    """(N,2,4) f32 -> (N, P+1, 2) f32 polyline points in pixel units,
    mirroring reference.curve_to_stroke in float32."""
    W = _bezier_weights_f32(P)
    s = strokes.astype(np.float32)
    pts, derivs = s[:, :, :2], s[:, :, 2:]
    before = pts - derivs
    after = pts + derivs
    p1, p2, p3, p4 = pts[:, :-1], after[:, :-1], before[:, 1:], pts[:, 1:]
    cp = np.stack([p1, p2, p3, p4], axis=3)          # (N, 1, 2, 4)
    sp = np.einsum("nsdk,kp->nspd", cp, W).astype(np.float32)  # (N,1,P,2)
    sp = sp.reshape(s.shape[0], -1, 2)
    poly = np.concatenate([sp, pts[:, -1:, :]], axis=1).astype(np.float32)
    return poly * np.float32(G)


def _band_clip(v, w, pad, x0, x1):
    """Clip segment v->w (f64) to row band [x0-pad, x1+pad]; return padded,
    canvas-clamped column range [c0, c1] or None."""
    lo_x, hi_x = x0 - pad, x1 + pad
    dx = w[0] - v[0]
    if abs(dx) < 1e-12:
        if v[0] < lo_x or v[0] > hi_x:
            return None
        s0, s1 = 0.0, 1.0
    else:
        sa = (lo_x - v[0]) / dx
        sb = (hi_x - v[0]) / dx
        s0 = max(0.0, min(sa, sb))
        s1 = min(1.0, max(sa, sb))
        if s0 > s1:
            return None
    ya = v[1] + s0 * (w[1] - v[1])
    yb = v[1] + s1 * (w[1] - v[1])
    c0 = max(0.0, min(ya, yb) - pad)
    c1 = min(G - 1.0, max(ya, yb) + pad)
    if c1 < c0:
        return None
    return int(np.floor(c0)), int(np.ceil(c1))


def _build_worklists(strokes, thicknesses, colors):
    """Returns (blocks_of_core, items_per_core, t, col) where each
    items_per_core[c] is a list of (n, v(2,), w(2,), c0)."""
    poly = _polylines(strokes).astype(np.float64)          # (N, P+1, 2)
    t = np.maximum(thicknesses.astype(np.float32) * np.float32(2.0)
                   + np.float32(0.5), np.float32(0.5))[:, 0]  # f32 (N,)
    col = np.clip(colors.astype(np.float32), 0.0, 1.0)     # (N, 3)
    r = 2.0 * t.astype(np.float64)
    pad = r + 1.0

    items_by_block = [[] for _ in range(NB)]
    cost = np.zeros(NB)
    for n in range(N):
        for i in range(P):
            v = poly[n, i]
            w = poly[n, i + 1]
            for b in range(NB):
                clip = _band_clip(v, w, pad[n], BH * b, BH * b + BH - 1)
                if clip is None:
                    continue
                c0, c1 = clip
                # windows of at most MAX_CLASS chunks
                width = c1 - c0 + 1
                cstart = c0
                while width > 0:
                    nch = min(MAX_CLASS, int(np.ceil(width / W_ITEM)))
                    cc = max(0, min(cstart, G - W_ITEM * nch))
                    items_by_block[b].append((n, v, w, cc, nch))
                    cstart += W_ITEM * nch
                    width -= W_ITEM * nch
                    cost[b] += nch

    order = np.argsort(-cost)
    loads = np.zeros(N_CORES)
    blocks_of = [[] for _ in range(N_CORES)]
    for b in order:
        cands = [c for c in range(N_CORES) if len(blocks_of[c]) < BLOCKS_PER_CORE]
        c = min(cands, key=lambda c: loads[c])
        blocks_of[c].append(int(b))
        loads[c] += cost[b]
    for c in range(N_CORES):
        blocks_of[c].sort()

    items_per_core = [
        [it for b in blocks_of[c] for it in items_by_block[b]]
        for c in range(N_CORES)
    ]
    return blocks_of, items_per_core, t, col


def _build_tables(blocks_of, windows_per_core, t, col, class_counts):
    """Build per-core input tables. Windows are (n, v, w, c0, nch); each core's
    windows are grouped by chunk-class and padded to the shared class_counts.
    Returns (in_maps, nitems, nwin)."""
    nitems = sum(cc * (ci + 1) for ci, cc in enumerate(class_counts))
    nwin = sum(class_counts)
    in_maps = []
    for c in range(N_CORES):
        by_class = [[] for _ in range(MAX_CLASS)]
        for win in windows_per_core[c]:
            by_class[win[4] - 1].append(win)
        ordered = []
        for ci in range(MAX_CLASS):
            assert len(by_class[ci]) <= class_counts[ci]
            pads = class_counts[ci] - len(by_class[ci])
            ordered += by_class[ci]
            ordered += [None] * pads

        # expand windows into chunk-items
        vx = np.zeros(nitems); vy = np.zeros(nitems)
        wx = np.zeros(nitems); wy = np.zeros(nitems)
        cstart = np.zeros(nitems, np.int64)
        i2t = np.full(nitems, 1.0)
        cols = np.zeros((nitems, 3))
        valid = np.zeros(nitems, bool)
        offv = np.zeros(nwin, np.int64)
        j = 0
        for widx, win in enumerate(ordered):
            if win is None:
                nch = _class_of(widx, class_counts)
                j += nch
                continue
            n, v, w, c0, nch = win
            offv[widx] = 3 * c0
            for i in range(nch):
                vx[j], vy[j] = v
                wx[j], wy[j] = w
                cstart[j] = c0 + W_ITEM * i
                i2t[j] = 1.0 / (2.0 * np.float64(t[n]))
                cols[j] = col[n]
                valid[j] = True
                j += 1
        assert j == nitems

        dx = wx - vx
        dy = wy - vy
        L = np.hypot(dx, dy)
        safe = L > 1e-9
        taux = np.where(safe, dx / np.where(safe, L, 1.0), 1.0)
        tauy = np.where(safe, dy / np.where(safe, L, 1.0), 0.0)
        Leff = np.where(safe, L, 0.0)
        nux = -tauy
        nuy = taux

        av = vx * taux + vy * tauy
        bv = vx * nux + vy * nuy
        ycols = cstart[:, None] + np.arange(W_ITEM)[None, :]   # (nitems, 32)
        # shifted-center tangent coord and normal coord, in 2t units
        a1 = taux * i2t
        b1 = nux * i2t
        a2 = (ycols * tauy[:, None] - (av + Leff / 2.0)[:, None]) * i2t[:, None]
        b2 = (ycols * nuy[:, None] - bv[:, None]) * i2t[:, None]
        hh = (Leff / 2.0) * i2t

        dead = ~valid
        a1[dead] = 0.0; b1[dead] = 0.0; hh[dead] = 0.0
        a2[dead] = 0.0; b2[dead] = 0.0
        cols[dead] = 0.0

        packw = nitems * W_ITEM
        rt = np.zeros((4, packw), np.float32)
        rt[0] = a2.ravel().astype(np.float32)
        rt[1] = np.repeat(a1, W_ITEM).astype(np.float32)
        rt[2] = b2.ravel().astype(np.float32)
        rt[3] = np.repeat(b1, W_ITEM).astype(np.float32)
        rh = hh.astype(np.float32).reshape(1, nitems)
        rci = cols.astype(np.float32).reshape(1, 3 * nitems)  # interleaved
        off = offv.astype(np.int32).reshape(1, nwin)

        xs = np.zeros(128, np.float32)
        for half, b in enumerate(blocks_of[c]):
            xs[half * BH:(half + 1) * BH] = BH * b + np.arange(BH)
        xt = np.zeros((66, 128), np.float32)
        for base in (0, 32, 64):
            xt[base] = 1.0
            xt[base + 1] = xs

        in_maps.append({"xt": xt, "rt": rt, "rh": rh, "rci": rci, "off": off})
    return in_maps


def _class_of(widx, class_counts):
    for ci, cc in enumerate(class_counts):
        if widx < cc:
            return ci + 1
        widx -= cc
    raise IndexError


# ---------------------------------------------------------------- bass side

def _build_program(nitems, class_counts):
    import concourse.bacc as bacc
    import concourse.mybir as mybir
    import concourse.bass as bass
    from concourse import tile

    f32 = mybir.dt.float32
    packw = nitems * W_ITEM
    nchunks = packw // CHUNK
    nwin = sum(class_counts)
    assert nchunks * CHUNK == packw

    nc = bacc.Bacc("TRN2", target_bir_lowering=False, debug=False,
                   num_devices=N_CORES)
    xt_d = nc.dram_tensor("xt", [66, 128], f32, kind="ExternalInput").ap()
    rt_d = nc.dram_tensor("rt", [4, packw], f32, kind="ExternalInput").ap()
    rh_d = nc.dram_tensor("rh", [1, nitems], f32, kind="ExternalInput").ap()
    rci_d = nc.dram_tensor("rci", [1, 3 * nitems], f32,
                           kind="ExternalInput").ap()
    off_d = nc.dram_tensor("off", [1, nwin], mybir.dt.int32,
                           kind="ExternalInput").ap()
    out_d = nc.dram_tensor("out", [128, 3 * G], f32, kind="ExternalOutput").ap()

    AF = mybir.ActivationFunctionType
    OP = mybir.AluOpType

    with tile.TileContext(nc) as tc:
        with (
            tc.tile_pool(name="const", bufs=1) as constp,
            tc.tile_pool(name="work", bufs=3) as workp,
            tc.tile_pool(name="psum", bufs=8, space="PSUM") as psump,
        ):
            # matmul operand pairs must sit at base partitions 0/32/64,
            # matching between lhsT and rhs
            xt = constp.tile([66, 128], f32)
            nc.sync.dma_start(xt[:], xt_d[:])
            rt = constp.tile([34, packw], f32)
            nc.sync.dma_start(rt[0:2, :], rt_d[0:2, :])
            nc.sync.dma_start(rt[32:34, :], rt_d[2:4, :])
            rh = constp.tile([1, nitems], f32)
            nc.sync.dma_start(rh[:], rh_d[:])
            rci = constp.tile([1, 3 * nitems], f32)
            nc.sync.dma_start(rci[:], rci_d[:])
            off = constp.tile([1, nwin], mybir.dt.int32)
            nc.sync.dma_start(off[:], off_d[:])

            # per-item H (half-length) and per-item interleaved color tables
            htab = constp.tile([128, nitems], f32)
            for co in range(0, nitems, CHUNK):
                ce = min(co + CHUNK, nitems)
                ph = psump.tile([128, ce - co], f32, tag="ps")
                nc.tensor.matmul(ph[:], xt[0:1, :], rh[0:1, co:ce])
                nc.scalar.copy(htab[:, co:ce], ph[:])
            coltab = constp.tile([128, 3 * nitems], f32)
            for co in range(0, 3 * nitems, CHUNK):
                ce = min(co + CHUNK, 3 * nitems)
                pc = psump.tile([128, ce - co], f32, tag="ps")
                nc.tensor.matmul(pc[:], xt[0:1, :], rci[0:1, co:ce])
                nc.scalar.copy(coltab[:, co:ce], pc[:])

            grid = constp.tile([128, 3 * G], f32)
            nc.gpsimd.memset(grid[:], 0.0)

            # vint: channel-interleaved packed values (c fastest)
            vint = constp.tile([128, 3 * packw], f32)

            for ch in range(nchunks):
                sl = slice(ch * CHUNK, (ch + 1) * CHUNK)
                k0 = ch * ITEMS_PER_CHUNK
                k1 = (ch + 1) * ITEMS_PER_CHUNK
                pa = psump.tile([128, CHUNK], f32, tag="ps")
                pb = psump.tile([128, CHUNK], f32, tag="ps")

                nc.tensor.matmul(pa[:], xt[0:2, :], rt[0:2, sl])
                nc.tensor.matmul(pb[:], xt[32:34, :], rt[32:34, sl])

                a1t = workp.tile([128, CHUNK], f32, tag="a1t")
                td = workp.tile([128, CHUNK], f32, tag="td")
                so = workp.tile([128, CHUNK], f32, tag="so")
                sb = workp.tile([128, CHUNK], f32, tag="sb")
                d2 = workp.tile([128, CHUNK], f32, tag="d2")
                dd = workp.tile([128, CHUNK], f32, tag="dd")
                m = workp.tile([128, CHUNK], f32, tag="m")

                # |a_centered| - H, overshoot^2 = relu(td)*td
                nc.scalar.activation(a1t[:], pa[:], AF.Abs)
                a3 = a1t[:].rearrange("p (k r) -> p k r", r=W_ITEM)
                hexp = htab[:, k0:k1].to_broadcast(
                    (128, ITEMS_PER_CHUNK, W_ITEM))
                td3 = td[:].rearrange("p (k r) -> p k r", r=W_ITEM)
                nc.vector.tensor_tensor(td3, a3, hexp, op=OP.subtract)
                nc.vector.scalar_tensor_tensor(
                    so[:], td[:], 0.0, td[:], op0=OP.max, op1=OP.mult)
                nc.scalar.activation(sb[:], pb[:], AF.Square)
                nc.gpsimd.tensor_tensor(d2[:], so[:], sb[:], op=OP.add)
                nc.scalar.activation(dd[:], d2[:], AF.Sqrt)
                nc.scalar.activation(m[:], dd[:], AF.Copy, bias=-1.0)

                # v_c = m * col_c, channel-interleaved
                vsl = vint[:, 3 * CHUNK * ch: 3 * CHUNK * (ch + 1)]
                v4 = vsl.rearrange("p (k r c) -> p k r c", r=W_ITEM, c=3)
                m4 = m[:].rearrange("p (k r) -> p k r", r=W_ITEM) \
                    .to_broadcast((128, ITEMS_PER_CHUNK, W_ITEM, 3))
                c4 = coltab[:, 3 * k0: 3 * k1].rearrange(
                    "p (k one c) -> p k one c", one=1, c=3) \
                    .to_broadcast((128, ITEMS_PER_CHUNK, W_ITEM, 3))
                nc.vector.tensor_tensor(v4, m4, c4, op=OP.mult)

            # scatter: per class, min-composite window spans into the canvas
            BATCH = 16
            widx = 0
            pk = 0
            for ci in range(MAX_CLASS):
                wspan = 3 * W_ITEM * (ci + 1)
                cls_n = class_counts[ci]
                done = 0
                while done < cls_n:
                    cnt = min(BATCH, cls_n - done)
                    _, vals = nc.values_load_multi_w_load_instructions(
                        off[0:1, widx:widx + cnt],
                        engines=[nc.vector.engine],
                        min_val=0,
                        max_val=3 * (G - W_ITEM * (ci + 1)),
                        skip_runtime_bounds_check=True,
                    )
                    for val in vals:
                        dst = grid[:, bass.ds(val, wspan)]
                        src = vint[:, 3 * W_ITEM * pk:
                                   3 * W_ITEM * pk + wspan]
                        nc.vector.tensor_tensor(dst, dst, src, op=OP.min)
                        pk += ci + 1
                        widx += 1
                    done += cnt
            assert pk == nitems and widx == nwin

            # negate + store
            outt = constp.tile([128, 3 * G], f32)
            for piece in range(4):
                slp = slice(piece * 3 * G // 4, (piece + 1) * 3 * G // 4)
                nc.scalar.activation(outt[:, slp], grid[:, slp],
                                     AF.Copy, scale=-1.0)
                nc.sync.dma_start(out_d[:, slp], outt[:, slp])

    nc.compile()
    return nc


# ---------------------------------------------------------------- entry

def _prepare(strokes, thicknesses, colors):
    blocks_of, windows_per_core, t, col = _build_worklists(
        strokes, thicknesses, colors)
    class_counts = [0] * MAX_CLASS
    for c in range(N_CORES):
        per = [0] * MAX_CLASS
        for win in windows_per_core[c]:
            per[win[4] - 1] += 1
        for ci in range(MAX_CLASS):
            class_counts[ci] = max(class_counts[ci], per[ci])
    # pad class-1 count so total chunk-items is a multiple of ITEMS_PER_CHUNK
    total = sum(cc * (ci + 1) for ci, cc in enumerate(class_counts))
    rem = (-total) % ITEMS_PER_CHUNK
    class_counts[0] += rem
    class_counts = tuple(class_counts)
    in_maps = _build_tables(blocks_of, windows_per_core, t, col, class_counts)
    nitems = sum(cc * (ci + 1) for ci, cc in enumerate(class_counts))
    return blocks_of, in_maps, nitems, class_counts


def kernel(strokes, thicknesses, colors):
    _install_ntff_hook()
    from concourse.bass_utils import run_bass_kernel_spmd

    strokes = np.asarray(strokes)
    thicknesses = np.asarray(thicknesses)
    colors = np.asarray(colors)

    blocks_of, in_maps, nitems, class_counts = _prepare(
        strokes, thicknesses, colors)
    key = (nitems, class_counts)
    if key not in _PROG_CACHE:
        _PROG_CACHE[key] = _build_program(nitems, class_counts)
    nc = _PROG_CACHE[key]

    res = run_bass_kernel_spmd(nc, in_maps, list(range(N_CORES)))

    out = np.zeros((3, G, G), np.float32)
    for c in range(N_CORES):
        o = res.results[c]["out"].reshape(128, G, 3)     # (y, c) interleaved
        for half, b in enumerate(blocks_of[c]):
            rows = o[half * BH:(half + 1) * BH]          # (64, 1024, 3)
            out[:, BH * b:BH * (b + 1), :] = rows.transpose(2, 0, 1)
    return out


if __name__ == "__main__":
    rng = np.random.default_rng(0)
    s = rng.random((N, 2, 4), np.float32)
    th = rng.random((N, 1), np.float32)
    co = rng.random((N, 3), np.float32)
    g = kernel(s, th, co)
    print("out", g.shape, g.dtype, g.min(), g.max())


# revision 18
# speedup vs baseline: 1.1033x; 1.1033x over previous
"""Bezier stroke renderer on 8 Trainium2 NeuronCores (Bass/Tile SPMD kernel).

Reference semantics: 32 cubic-Bezier strokes, each sampled into a 16-segment
polyline, rasterized onto a 1024x1024 canvas: per pixel and segment,
darkness = clip((2t - dist_to_segment)/(2t), 0, 1), max over segments within a
stroke, then grid = max(grid, darkness * color) over strokes (3 channels).

Strategy (sharding: spatial split of the pixel grid by rows):
  - The canvas is split into 16 blocks of 64 rows; each core owns 2 blocks
    (greedy-balanced by estimated work), giving a [128 partitions x 1024 cols]
    canvas tile per core held in SBUF.
  - Only pixels within 2t+1 of a segment can be painted, so host code builds a
    worklist of (segment, block) windows, chunked into fixed 32-column items.
    All per-item parameters are shipped as per-core data tables; the single
    SPMD instruction stream is identical across cores (counts padded to the
    max over cores).
  - Distance math in the segment's tangent/normal frame, pre-scaled by 1/(2t):
        dist/(2t) = sqrt(relu(a-L)^2 + min(a,0)^2 + b^2)
    where a,b are affine in pixel coords -> computed by TensorE matmuls
    (lhsT = [x_p; 1], per-column coefficients from host tables).
  - Per channel, w_c = (dist/(2t) - 1) * col_c is min-composited into a
    negated-grid accumulator via register-offset dynamic windows (scatter),
    split across the DVE and GPSIMD engines with separate accumulators.
  - Final: out = -min(grid_dve, grid_gps), DMA to DRAM.
"""

import sys
import types
import contextlib
import ctypes

sys.path.insert(0, "/opt/trn_rl_repo")

import numpy as np

G = 1024
P = 16
N = 32
N_CORES = 8
BH = 64           # block height (rows)
NB = G // BH      # 16 blocks
BLOCKS_PER_CORE = NB // N_CORES
W_ITEM = 32       # columns per packed chunk-item
MAX_CLASS = 2     # scatter windows are 1..MAX_CLASS chunk-items wide
CHUNK = 512       # packed columns per matmul/PSUM chunk
ITEMS_PER_CHUNK = CHUNK // W_ITEM  # 16

_PROG_CACHE = {}
_HOOK_INSTALLED = False


def _install_ntff_hook():
    """Register the NTFF profile hook (mirrors trn_boot.py) so
    run_bass_kernel_spmd(trace=True) can measure HW exec time."""
    global _HOOK_INSTALLED
    if _HOOK_INSTALLED:
        return
    _HOOK_INSTALLED = True
    try:
        import antenv
        mod = types.ModuleType("antenv.axon_hooks")
        holder = [None]
        mod.set_axon_ntff_profile_hook = lambda h: holder.__setitem__(0, h)
        mod.get_axon_ntff_profile_hook = lambda: holder[0]
        sys.modules["antenv.axon_hooks"] = mod
        antenv.axon_hooks = mod

        lib = ctypes.CDLL("/opt/axon/libaxon_pjrt.so")
        if not hasattr(lib, "axon_start_nrt_profile"):
            return
        lib.axon_start_nrt_profile.argtypes = [
            ctypes.POINTER(ctypes.c_int64),
            ctypes.c_size_t,
        ]
        lib.axon_start_nrt_profile.restype = ctypes.c_int64
        lib.axon_stop_nrt_profile.argtypes = [ctypes.c_char_p]
        lib.axon_stop_nrt_profile.restype = ctypes.c_int64

        @contextlib.contextmanager
        def _hook(output_dir, device_ids):
            import jax
            jax.devices()
            if device_ids:
                ids = (ctypes.c_int64 * len(device_ids))(*device_ids)
                rc = lib.axon_start_nrt_profile(ids, len(device_ids))
            else:
                rc = lib.axon_start_nrt_profile(None, 0)
            if rc != 0:
                raise RuntimeError(f"axon_start_nrt_profile rc={rc}")
            try:
                yield
            finally:
                n = lib.axon_stop_nrt_profile(str(output_dir).encode())
                print(f"profile: {n} file(s) written to {output_dir}",
                      file=sys.stderr)

        mod.set_axon_ntff_profile_hook(_hook)
    except Exception:
        pass


# ---------------------------------------------------------------- host side

def _bezier_weights_f32(p):
    t = np.arange(p, dtype=np.float64)
    w1 = (p - t) ** 3 / p ** 3
    w2 = 3 * (p - t) ** 2 * t / p ** 3
    w3 = 3 * (p - t) * t ** 2 / p ** 3
    w4 = t ** 3 / p ** 3
    return np.stack([w1, w2, w3, w4]).astype(np.float32)  # (4, P)


def _polylines(strokes):
    """(N,2,4) f32 -> (N, P+1, 2) f32 polyline points in pixel units,
    mirroring reference.curve_to_stroke in float32."""
    W = _bezier_weights_f32(P)
    s = strokes.astype(np.float32)
    pts, derivs = s[:, :, :2], s[:, :, 2:]
    before = pts - derivs
    after = pts + derivs
    p1, p2, p3, p4 = pts[:, :-1], after[:, :-1], before[:, 1:], pts[:, 1:]
    cp = np.stack([p1, p2, p3, p4], axis=3)          # (N, 1, 2, 4)
    sp = np.einsum("nsdk,kp->nspd", cp, W).astype(np.float32)  # (N,1,P,2)
    sp = sp.reshape(s.shape[0], -1, 2)
    poly = np.concatenate([sp, pts[:, -1:, :]], axis=1).astype(np.float32)
    return poly * np.float32(G)


def _band_clip(v, w, pad, x0, x1):
    """Clip segment v->w (f64) to row band [x0-pad, x1+pad]; return padded,
    canvas-clamped column range [c0, c1] or None."""
    lo_x, hi_x = x0 - pad, x1 + pad
    dx = w[0] - v[0]
    if abs(dx) < 1e-12:
        if v[0] < lo_x or v[0] > hi_x:
            return None
        s0, s1 = 0.0, 1.0
    else:
        sa = (lo_x - v[0]) / dx
        sb = (hi_x - v[0]) / dx
        s0 = max(0.0, min(sa, sb))
        s1 = min(1.0, max(sa, sb))
        if s0 > s1:
            return None
    ya = v[1] + s0 * (w[1] - v[1])
    yb = v[1] + s1 * (w[1] - v[1])
    c0 = max(0.0, min(ya, yb) - pad)
    c1 = min(G - 1.0, max(ya, yb) + pad)
    if c1 < c0:
        return None
    return int(np.floor(c0)), int(np.ceil(c1))


def _build_worklists(strokes, thicknesses, colors):
    """Returns (blocks_of_core, items_per_core, t, col) where each
    items_per_core[c] is a list of (n, v(2,), w(2,), c0)."""
    poly = _polylines(strokes).astype(np.float64)          # (N, P+1, 2)
    t = np.maximum(thicknesses.astype(np.float32) * np.float32(2.0)
                   + np.float32(0.5), np.float32(0.5))[:, 0]  # f32 (N,)
    col = np.clip(colors.astype(np.float32), 0.0, 1.0)     # (N, 3)
    r = 2.0 * t.astype(np.float64)
    pad = r + 1.0

    items_by_block = [[] for _ in range(NB)]
    cost = np.zeros(NB)
    for n in range(N):
        for i in range(P):
            v = poly[n, i]
            w = poly[n, i + 1]
            for b in range(NB):
                clip = _band_clip(v, w, pad[n], BH * b, BH * b + BH - 1)
                if clip is None:
                    continue
                c0, c1 = clip
                # windows of at most MAX_CLASS chunks
                width = c1 - c0 + 1
                cstart = c0
                while width > 0:
                    nch = min(MAX_CLASS, int(np.ceil(width / W_ITEM)))
                    cc = max(0, min(cstart, G - W_ITEM * nch))
                    items_by_block[b].append((n, v, w, cc, nch))
                    cstart += W_ITEM * nch
                    width -= W_ITEM * nch
                    cost[b] += nch

    order = np.argsort(-cost)
    loads = np.zeros(N_CORES)
    blocks_of = [[] for _ in range(N_CORES)]
    for b in order:
        cands = [c for c in range(N_CORES) if len(blocks_of[c]) < BLOCKS_PER_CORE]
        c = min(cands, key=lambda c: loads[c])
        blocks_of[c].append(int(b))
        loads[c] += cost[b]
    for c in range(N_CORES):
        blocks_of[c].sort()

    items_per_core = [
        [it for b in blocks_of[c] for it in items_by_block[b]]
        for c in range(N_CORES)
    ]
    return blocks_of, items_per_core, t, col


def _build_tables(blocks_of, windows_per_core, t, col, class_counts):
    """Build per-core input tables. Windows are (n, v, w, c0, nch); each core's
    windows are grouped by chunk-class and padded to the shared class_counts.
    Returns (in_maps, nitems, nwin)."""
    nitems = sum(cc * (ci + 1) for ci, cc in enumerate(class_counts))
    nwin = sum(class_counts)
    in_maps = []
    for c in range(N_CORES):
        by_class = [[] for _ in range(MAX_CLASS)]
        for win in windows_per_core[c]:
            by_class[win[4] - 1].append(win)
        ordered = []
        for ci in range(MAX_CLASS):
            assert len(by_class[ci]) <= class_counts[ci]
            pads = class_counts[ci] - len(by_class[ci])
            ordered += by_class[ci]
            ordered += [None] * pads

        # expand windows into chunk-items
        vx = np.zeros(nitems); vy = np.zeros(nitems)
        wx = np.zeros(nitems); wy = np.zeros(nitems)
        cstart = np.zeros(nitems, np.int64)
        i2t = np.full(nitems, 1.0)
        cols = np.zeros((nitems, 3))
        valid = np.zeros(nitems, bool)
        offv = np.zeros(nwin, np.int64)
        j = 0
        for widx, win in enumerate(ordered):
            if win is None:
                nch = _class_of(widx, class_counts)
                j += nch
                continue
            n, v, w, c0, nch = win
            offv[widx] = 3 * c0
            for i in range(nch):
                vx[j], vy[j] = v
                wx[j], wy[j] = w
                cstart[j] = c0 + W_ITEM * i
                i2t[j] = 1.0 / (2.0 * np.float64(t[n]))
                cols[j] = col[n]
                valid[j] = True
                j += 1
        assert j == nitems

        dx = wx - vx
        dy = wy - vy
        L = np.hypot(dx, dy)
        safe = L > 1e-9
        taux = np.where(safe, dx / np.where(safe, L, 1.0), 1.0)
        tauy = np.where(safe, dy / np.where(safe, L, 1.0), 0.0)
        Leff = np.where(safe, L, 0.0)
        nux = -tauy
        nuy = taux

        av = vx * taux + vy * tauy
        bv = vx * nux + vy * nuy
        ycols = cstart[:, None] + np.arange(W_ITEM)[None, :]   # (nitems, 32)
        # shifted-center tangent coord and normal coord, in 2t units
        a1 = taux * i2t
        b1 = nux * i2t
        a2 = (ycols * tauy[:, None] - (av + Leff / 2.0)[:, None]) * i2t[:, None]
        b2 = (ycols * nuy[:, None] - bv[:, None]) * i2t[:, None]
        hh = (Leff / 2.0) * i2t

        dead = ~valid
        a1[dead] = 0.0; b1[dead] = 0.0; hh[dead] = 0.0
        a2[dead] = 0.0; b2[dead] = 0.0
        cols[dead] = 0.0

        packw = nitems * W_ITEM
        # a-centered rows and (a-centered - H)/(-a-centered - H) trick are not
        # needed: ship plain tangent rows plus shifted rows for the two relus
        a2u = a2 + hh[:, None]          # tangent coord from segment start
        rt = np.zeros((6, packw), np.float32)
        rt[0] = (a2u - 2.0 * hh[:, None]).ravel().astype(np.float32)  # a - L
        rt[1] = np.repeat(a1, W_ITEM).astype(np.float32)
        rt[2] = a2u.ravel().astype(np.float32)                        # a
        rt[3] = rt[1]
        rt[4] = b2.ravel().astype(np.float32)
        rt[5] = np.repeat(b1, W_ITEM).astype(np.float32)
        rc = np.stack([
            np.repeat(cols[:, 0], W_ITEM),
            np.repeat(cols[:, 1], W_ITEM),
            np.repeat(cols[:, 2], W_ITEM),
        ]).astype(np.float32)
        off = offv.astype(np.int32).reshape(1, nwin)

        xs = np.zeros(128, np.float32)
        for half, b in enumerate(blocks_of[c]):
            xs[half * BH:(half + 1) * BH] = BH * b + np.arange(BH)
        xt = np.zeros((66, 128), np.float32)
        for base in (0, 32, 64):
            xt[base] = 1.0
            xt[base + 1] = xs

        in_maps.append({"xt": xt, "rt": rt, "rc": rc, "off": off})
    return in_maps


def _class_of(widx, class_counts):
    for ci, cc in enumerate(class_counts):
        if widx < cc:
            return ci + 1
        widx -= cc
    raise IndexError


# ---------------------------------------------------------------- bass side

def _build_program(nitems, class_counts):
    import concourse.bacc as bacc
    import concourse.mybir as mybir
    import concourse.bass as bass
    from concourse import tile

    f32 = mybir.dt.float32
    packw = nitems * W_ITEM
    nchunks = packw // CHUNK
    nwin = sum(class_counts)
    assert nchunks * CHUNK == packw

    nc = bacc.Bacc("TRN2", target_bir_lowering=False, debug=False,
                   num_devices=N_CORES)
    xt_d = nc.dram_tensor("xt", [66, 128], f32, kind="ExternalInput").ap()
    rt_d = nc.dram_tensor("rt", [6, packw], f32, kind="ExternalInput").ap()
    rc_d = nc.dram_tensor("rc", [3, packw], f32, kind="ExternalInput").ap()
    off_d = nc.dram_tensor("off", [1, nwin], mybir.dt.int32,
                           kind="ExternalInput").ap()
    out_d = nc.dram_tensor("out", [128, 3 * G], f32, kind="ExternalOutput").ap()

    AF = mybir.ActivationFunctionType
    OP = mybir.AluOpType

    with tile.TileContext(nc) as tc:
        with (
            tc.tile_pool(name="const", bufs=1) as constp,
            tc.tile_pool(name="work", bufs=3) as workp,
            tc.tile_pool(name="psum", bufs=8, space="PSUM") as psump,
        ):
            # matmul operand pairs must sit at base partitions 0/32/64,
            # matching between lhsT and rhs
            xt = constp.tile([66, 128], f32)
            nc.sync.dma_start(xt[:], xt_d[:])
            rt = constp.tile([66, packw], f32)
            nc.sync.dma_start(rt[0:2, :], rt_d[0:2, :])
            nc.sync.dma_start(rt[32:34, :], rt_d[2:4, :])
            nc.sync.dma_start(rt[64:66, :], rt_d[4:6, :])
            rc = constp.tile([65, packw], f32)
            nc.sync.dma_start(rc[0:1, :], rc_d[0:1, :])
            nc.sync.dma_start(rc[32:33, :], rc_d[1:2, :])
            nc.sync.dma_start(rc[64:65, :], rc_d[2:3, :])
            off = constp.tile([1, nwin], mybir.dt.int32)
            nc.sync.dma_start(off[:], off_d[:])

            grid = constp.tile([128, 3 * G], f32)
            nc.gpsimd.memset(grid[:], 0.0)

            # vint: channel-interleaved packed values (c fastest)
            vint = constp.tile([128, 3 * packw], f32)
            vint3 = vint[:].rearrange("p (j c) -> p j c", c=3)

            for ch in range(nchunks):
                sl = slice(ch * CHUNK, (ch + 1) * CHUNK)
                pal = psump.tile([128, CHUNK], f32, tag="ps")
                pa = psump.tile([128, CHUNK], f32, tag="ps")
                pb = psump.tile([128, CHUNK], f32, tag="ps")
                pc0 = psump.tile([128, CHUNK], f32, tag="ps")
                pc1 = psump.tile([128, CHUNK], f32, tag="ps")
                pc2 = psump.tile([128, CHUNK], f32, tag="ps")

                nc.tensor.matmul(pal[:], xt[0:2, :], rt[0:2, sl])
                nc.tensor.matmul(pa[:], xt[32:34, :], rt[32:34, sl])
                nc.tensor.matmul(pb[:], xt[64:66, :], rt[64:66, sl])
                nc.tensor.matmul(pc0[:], xt[0:1, :], rc[0:1, sl])
                nc.tensor.matmul(pc1[:], xt[32:33, :], rc[32:33, sl])
                nc.tensor.matmul(pc2[:], xt[64:65, :], rc[64:65, sl])

                q1 = workp.tile([128, CHUNK], f32, tag="q1")
                q2 = workp.tile([128, CHUNK], f32, tag="q2")
                s1 = workp.tile([128, CHUNK], f32, tag="s1")
                s2 = workp.tile([128, CHUNK], f32, tag="s2")
                sb = workp.tile([128, CHUNK], f32, tag="sb")
                d2a = workp.tile([128, CHUNK], f32, tag="d2a")
                d2 = workp.tile([128, CHUNK], f32, tag="d2")
                dd = workp.tile([128, CHUNK], f32, tag="dd")

                # overshoot beyond segment end / before start, in 2t units
                nc.scalar.activation(q1[:], pal[:], AF.Relu)
                nc.scalar.activation(q2[:], pa[:], AF.Relu, scale=-1.0)
                nc.scalar.activation(s1[:], q1[:], AF.Square)
                nc.scalar.activation(s2[:], q2[:], AF.Square)
                nc.scalar.activation(sb[:], pb[:], AF.Square)
                nc.gpsimd.tensor_tensor(d2a[:], s1[:], s2[:], op=OP.add)
                nc.gpsimd.tensor_tensor(d2[:], d2a[:], sb[:], op=OP.add)
                nc.scalar.activation(dd[:], d2[:], AF.Sqrt)

                # w_c = (dd - 1) * col_c into channel-interleaved vint
                vch = vint3[:, sl, :]
                nc.vector.scalar_tensor_tensor(
                    vch[:, :, 0], dd[:], 1.0, pc0[:],
                    op0=OP.subtract, op1=OP.mult)
                nc.vector.scalar_tensor_tensor(
                    vch[:, :, 1], dd[:], 1.0, pc1[:],
                    op0=OP.subtract, op1=OP.mult)
                nc.vector.scalar_tensor_tensor(
                    vch[:, :, 2], dd[:], 1.0, pc2[:],
                    op0=OP.subtract, op1=OP.mult)

            # scatter: per class, min-composite window spans into the canvas
            BATCH = 8
            widx = 0
            pk = 0
            for ci in range(MAX_CLASS):
                wspan = 3 * W_ITEM * (ci + 1)
                cls_n = class_counts[ci]
                done = 0
                while done < cls_n:
                    cnt = min(BATCH, cls_n - done)
                    _, vals = nc.values_load_multi_w_load_instructions(
                        off[0:1, widx:widx + cnt],
                        engines=[nc.vector.engine],
                        min_val=0,
                        max_val=3 * (G - W_ITEM * (ci + 1)),
                        skip_runtime_bounds_check=True,
                    )
                    for val in vals:
                        dst = grid[:, bass.ds(val, wspan)]
                        src = vint[:, 3 * W_ITEM * pk:
                                   3 * W_ITEM * pk + wspan]
                        nc.vector.tensor_tensor(dst, dst, src, op=OP.min)
                        pk += ci + 1
                        widx += 1
                    done += cnt
            assert pk == nitems and widx == nwin

            # negate + store
            outt = constp.tile([128, 3 * G], f32)
            for piece in range(4):
                slp = slice(piece * 3 * G // 4, (piece + 1) * 3 * G // 4)
                nc.scalar.activation(outt[:, slp], grid[:, slp],
                                     AF.Copy, scale=-1.0)
                nc.sync.dma_start(out_d[:, slp], outt[:, slp])

    nc.compile()
    return nc


# ---------------------------------------------------------------- entry

def _prepare(strokes, thicknesses, colors):
    blocks_of, windows_per_core, t, col = _build_worklists(
        strokes, thicknesses, colors)
    class_counts = [0] * MAX_CLASS
    for c in range(N_CORES):
        per = [0] * MAX_CLASS
        for win in windows_per_core[c]:
            per[win[4] - 1] += 1
        for ci in range(MAX_CLASS):
            class_counts[ci] = max(class_counts[ci], per[ci])
    # pad class-1 count so total chunk-items is a multiple of ITEMS_PER_CHUNK
    total = sum(cc * (ci + 1) for ci, cc in enumerate(class_counts))
    rem = (-total) % ITEMS_PER_CHUNK
    class_counts[0] += rem
    class_counts = tuple(class_counts)
    in_maps = _build_tables(blocks_of, windows_per_core, t, col, class_counts)
    nitems = sum(cc * (ci + 1) for ci, cc in enumerate(class_counts))
    return blocks_of, in_maps, nitems, class_counts


def kernel(strokes, thicknesses, colors):
    _install_ntff_hook()
    from concourse.bass_utils import run_bass_kernel_spmd

    strokes = np.asarray(strokes)
    thicknesses = np.asarray(thicknesses)
    colors = np.asarray(colors)

    blocks_of, in_maps, nitems, class_counts = _prepare(
        strokes, thicknesses, colors)
    key = (nitems, class_counts)
    if key not in _PROG_CACHE:
        _PROG_CACHE[key] = _build_program(nitems, class_counts)
    nc = _PROG_CACHE[key]

    res = run_bass_kernel_spmd(nc, in_maps, list(range(N_CORES)))

    out = np.zeros((3, G, G), np.float32)
    for c in range(N_CORES):
        o = res.results[c]["out"].reshape(128, G, 3)     # (y, c) interleaved
        for half, b in enumerate(blocks_of[c]):
            rows = o[half * BH:(half + 1) * BH]          # (64, 1024, 3)
            out[:, BH * b:BH * (b + 1), :] = rows.transpose(2, 0, 1)
    return out


if __name__ == "__main__":
    rng = np.random.default_rng(0)
    s = rng.random((N, 2, 4), np.float32)
    th = rng.random((N, 1), np.float32)
    co = rng.random((N, 3), np.float32)
    g = kernel(s, th, co)
    print("out", g.shape, g.dtype, g.min(), g.max())


# revision 20
# speedup vs baseline: 1.2222x; 1.1077x over previous
"""Bezier stroke renderer on 8 Trainium2 NeuronCores (Bass/Tile SPMD kernel).

Reference semantics: 32 cubic-Bezier strokes, each sampled into a 16-segment
polyline, rasterized onto a 1024x1024 canvas: per pixel and segment,
darkness = clip((2t - dist_to_segment)/(2t), 0, 1), max over segments within a
stroke, then grid = max(grid, darkness * color) over strokes (3 channels).

Strategy (sharding: spatial split of the pixel grid by rows):
  - The canvas is split into 16 blocks of 64 rows; each core owns 2 blocks
    (greedy-balanced by estimated work), giving a [128 partitions x 1024 cols]
    canvas tile per core held in SBUF.
  - Only pixels within 2t+1 of a segment can be painted, so host code builds a
    worklist of (segment, block) windows, chunked into fixed 32-column items.
    All per-item parameters are shipped as per-core data tables; the single
    SPMD instruction stream is identical across cores (counts padded to the
    max over cores).
  - Distance math in the segment's tangent/normal frame, pre-scaled by 1/(2t):
        dist/(2t) = sqrt(relu(a-L)^2 + min(a,0)^2 + b^2)
    where a,b are affine in pixel coords -> computed by TensorE matmuls
    (lhsT = [x_p; 1], per-column coefficients from host tables).
  - Per channel, w_c = (dist/(2t) - 1) * col_c is min-composited into a
    negated-grid accumulator via register-offset dynamic windows (scatter),
    split across the DVE and GPSIMD engines with separate accumulators.
  - Final: out = -min(grid_dve, grid_gps), DMA to DRAM.
"""

import sys
import types
import contextlib
import ctypes

sys.path.insert(0, "/opt/trn_rl_repo")

import numpy as np

G = 1024
P = 16
N = 32
N_CORES = 8
BH = 64           # block height (rows)
NB = G // BH      # 16 blocks
BLOCKS_PER_CORE = NB // N_CORES
W_ITEM = 32       # columns per packed chunk-item
MAX_CLASS = 2     # scatter windows are 1..MAX_CLASS chunk-items wide
CHUNK = 512       # packed columns per matmul/PSUM chunk
ITEMS_PER_CHUNK = CHUNK // W_ITEM  # 16

_PROG_CACHE = {}
_HOOK_INSTALLED = False


def _install_ntff_hook():
    """Register the NTFF profile hook (mirrors trn_boot.py) so
    run_bass_kernel_spmd(trace=True) can measure HW exec time."""
    global _HOOK_INSTALLED
    if _HOOK_INSTALLED:
        return
    _HOOK_INSTALLED = True
    try:
        import antenv
        mod = types.ModuleType("antenv.axon_hooks")
        holder = [None]
        mod.set_axon_ntff_profile_hook = lambda h: holder.__setitem__(0, h)
        mod.get_axon_ntff_profile_hook = lambda: holder[0]
        sys.modules["antenv.axon_hooks"] = mod
        antenv.axon_hooks = mod

        lib = ctypes.CDLL("/opt/axon/libaxon_pjrt.so")
        if not hasattr(lib, "axon_start_nrt_profile"):
            return
        lib.axon_start_nrt_profile.argtypes = [
            ctypes.POINTER(ctypes.c_int64),
            ctypes.c_size_t,
        ]
        lib.axon_start_nrt_profile.restype = ctypes.c_int64
        lib.axon_stop_nrt_profile.argtypes = [ctypes.c_char_p]
        lib.axon_stop_nrt_profile.restype = ctypes.c_int64

        @contextlib.contextmanager
        def _hook(output_dir, device_ids):
            import jax
            jax.devices()
            if device_ids:
                ids = (ctypes.c_int64 * len(device_ids))(*device_ids)
                rc = lib.axon_start_nrt_profile(ids, len(device_ids))
            else:
                rc = lib.axon_start_nrt_profile(None, 0)
            if rc != 0:
                raise RuntimeError(f"axon_start_nrt_profile rc={rc}")
            try:
                yield
            finally:
                n = lib.axon_stop_nrt_profile(str(output_dir).encode())
                print(f"profile: {n} file(s) written to {output_dir}",
                      file=sys.stderr)

        mod.set_axon_ntff_profile_hook(_hook)
    except Exception:
        pass


# ---------------------------------------------------------------- host side

def _bezier_weights_f32(p):
    t = np.arange(p, dtype=np.float64)
    w1 = (p - t) ** 3 / p ** 3
    w2 = 3 * (p - t) ** 2 * t / p ** 3
    w3 = 3 * (p - t) * t ** 2 / p ** 3
    w4 = t ** 3 / p ** 3
    return np.stack([w1, w2, w3, w4]).astype(np.float32)  # (4, P)


def _polylines(strokes):
    """(N,2,4) f32 -> (N, P+1, 2) f32 polyline points in pixel units,
    mirroring reference.curve_to_stroke in float32."""
    W = _bezier_weights_f32(P)
    s = strokes.astype(np.float32)
    pts, derivs = s[:, :, :2], s[:, :, 2:]
    before = pts - derivs
    after = pts + derivs
    p1, p2, p3, p4 = pts[:, :-1], after[:, :-1], before[:, 1:], pts[:, 1:]
    cp = np.stack([p1, p2, p3, p4], axis=3)          # (N, 1, 2, 4)
    sp = np.einsum("nsdk,kp->nspd", cp, W).astype(np.float32)  # (N,1,P,2)
    sp = sp.reshape(s.shape[0], -1, 2)
    poly = np.concatenate([sp, pts[:, -1:, :]], axis=1).astype(np.float32)
    return poly * np.float32(G)


def _band_clip(v, w, pad, x0, x1):
    """Clip segment v->w (f64) to row band [x0-pad, x1+pad]; return padded,
    canvas-clamped column range [c0, c1] or None."""
    lo_x, hi_x = x0 - pad, x1 + pad
    dx = w[0] - v[0]
    if abs(dx) < 1e-12:
        if v[0] < lo_x or v[0] > hi_x:
            return None
        s0, s1 = 0.0, 1.0
    else:
        sa = (lo_x - v[0]) / dx
        sb = (hi_x - v[0]) / dx
        s0 = max(0.0, min(sa, sb))
        s1 = min(1.0, max(sa, sb))
        if s0 > s1:
            return None
    ya = v[1] + s0 * (w[1] - v[1])
    yb = v[1] + s1 * (w[1] - v[1])
    c0 = max(0.0, min(ya, yb) - pad)
    c1 = min(G - 1.0, max(ya, yb) + pad)
    if c1 < c0:
        return None
    return int(np.floor(c0)), int(np.ceil(c1))


def _build_worklists(strokes, thicknesses, colors):
    """Returns (blocks_of_core, items_per_core, t, col) where each
    items_per_core[c] is a list of (n, v(2,), w(2,), c0)."""
    poly = _polylines(strokes).astype(np.float64)          # (N, P+1, 2)
    t = np.maximum(thicknesses.astype(np.float32) * np.float32(2.0)
                   + np.float32(0.5), np.float32(0.5))[:, 0]  # f32 (N,)
    col = np.clip(colors.astype(np.float32), 0.0, 1.0)     # (N, 3)
    r = 2.0 * t.astype(np.float64)
    pad = r + 1.0

    items_by_block = [[] for _ in range(NB)]
    cost = np.zeros(NB)
    for n in range(N):
        for i in range(P):
            v = poly[n, i]
            w = poly[n, i + 1]
            for b in range(NB):
                clip = _band_clip(v, w, pad[n], BH * b, BH * b + BH - 1)
                if clip is None:
                    continue
                c0, c1 = clip
                # windows of at most MAX_CLASS chunks
                width = c1 - c0 + 1
                cstart = c0
                while width > 0:
                    nch = min(MAX_CLASS, int(np.ceil(width / W_ITEM)))
                    cc = max(0, min(cstart, G - W_ITEM * nch))
                    items_by_block[b].append((n, v, w, cc, nch))
                    cstart += W_ITEM * nch
                    width -= W_ITEM * nch
                    cost[b] += nch

    order = np.argsort(-cost)
    loads = np.zeros(N_CORES)
    blocks_of = [[] for _ in range(N_CORES)]
    for b in order:
        cands = [c for c in range(N_CORES) if len(blocks_of[c]) < BLOCKS_PER_CORE]
        c = min(cands, key=lambda c: loads[c])
        blocks_of[c].append(int(b))
        loads[c] += cost[b]
    for c in range(N_CORES):
        blocks_of[c].sort()

    items_per_core = [
        [it for b in blocks_of[c] for it in items_by_block[b]]
        for c in range(N_CORES)
    ]
    return blocks_of, items_per_core, t, col


def _build_tables(blocks_of, windows_per_core, t, col, class_counts):
    """Build per-core input tables. Windows are (n, v, w, c0, nch); each core's
    windows are grouped by chunk-class and padded to the shared class_counts.
    Returns (in_maps, nitems, nwin)."""
    nitems = sum(cc * (ci + 1) for ci, cc in enumerate(class_counts))
    nwin = sum(class_counts)
    in_maps = []
    for c in range(N_CORES):
        by_class = [[] for _ in range(MAX_CLASS)]
        for win in windows_per_core[c]:
            by_class[win[4] - 1].append(win)
        ordered = []
        for ci in range(MAX_CLASS):
            assert len(by_class[ci]) <= class_counts[ci]
            pads = class_counts[ci] - len(by_class[ci])
            ordered += by_class[ci]
            ordered += [None] * pads

        # expand windows into chunk-items
        vx = np.zeros(nitems); vy = np.zeros(nitems)
        wx = np.zeros(nitems); wy = np.zeros(nitems)
        cstart = np.zeros(nitems, np.int64)
        i2t = np.full(nitems, 1.0)
        cols = np.zeros((nitems, 3))
        valid = np.zeros(nitems, bool)
        offv = np.zeros(nwin, np.int64)
        j = 0
        for widx, win in enumerate(ordered):
            if win is None:
                nch = _class_of(widx, class_counts)
                j += nch
                continue
            n, v, w, c0, nch = win
            offv[widx] = 3 * c0
            for i in range(nch):
                vx[j], vy[j] = v
                wx[j], wy[j] = w
                cstart[j] = c0 + W_ITEM * i
                i2t[j] = 1.0 / (2.0 * np.float64(t[n]))
                cols[j] = col[n]
                valid[j] = True
                j += 1
        assert j == nitems

        dx = wx - vx
        dy = wy - vy
        L = np.hypot(dx, dy)
        safe = L > 1e-9
        taux = np.where(safe, dx / np.where(safe, L, 1.0), 1.0)
        tauy = np.where(safe, dy / np.where(safe, L, 1.0), 0.0)
        Leff = np.where(safe, L, 0.0)
        nux = -tauy
        nuy = taux

        av = vx * taux + vy * tauy
        bv = vx * nux + vy * nuy
        ycols = cstart[:, None] + np.arange(W_ITEM)[None, :]   # (nitems, 32)
        # shifted-center tangent coord and normal coord, in 2t units
        a1 = taux * i2t
        b1 = nux * i2t
        a2 = (ycols * tauy[:, None] - (av + Leff / 2.0)[:, None]) * i2t[:, None]
        b2 = (ycols * nuy[:, None] - bv[:, None]) * i2t[:, None]
        hh = (Leff / 2.0) * i2t

        dead = ~valid
        a1[dead] = 0.0; b1[dead] = 0.0; hh[dead] = 0.0
        a2[dead] = 0.0; b2[dead] = 0.0
        cols[dead] = 0.0

        packw = nitems * W_ITEM
        # a-centered rows and (a-centered - H)/(-a-centered - H) trick are not
        # needed: ship plain tangent rows plus shifted rows for the two relus
        a2u = a2 + hh[:, None]          # tangent coord from segment start
        rt = np.zeros((6, packw), np.float32)
        rt[0] = (a2u - 2.0 * hh[:, None]).ravel().astype(np.float32)  # a - L
        rt[1] = np.repeat(a1, W_ITEM).astype(np.float32)
        rt[2] = a2u.ravel().astype(np.float32)                        # a
        rt[3] = rt[1]
        rt[4] = b2.ravel().astype(np.float32)
        rt[5] = np.repeat(b1, W_ITEM).astype(np.float32)
        rc = np.stack([
            np.repeat(cols[:, 0], W_ITEM),
            np.repeat(cols[:, 1], W_ITEM),
            np.repeat(cols[:, 2], W_ITEM),
        ]).astype(np.float32)
        off = offv.astype(np.int32).reshape(1, nwin)

        xs = np.zeros(128, np.float32)
        for half, b in enumerate(blocks_of[c]):
            xs[half * BH:(half + 1) * BH] = BH * b + np.arange(BH)
        xt = np.zeros((66, 128), np.float32)
        for base in (0, 32, 64):
            xt[base] = 1.0
            xt[base + 1] = xs

        in_maps.append({"xt": xt, "rt": rt, "rc": rc, "off": off})
    return in_maps


def _class_of(widx, class_counts):
    for ci, cc in enumerate(class_counts):
        if widx < cc:
            return ci + 1
        widx -= cc
    raise IndexError


# ---------------------------------------------------------------- bass side

def _build_program(nitems, class_counts):
    import concourse.bacc as bacc
    import concourse.mybir as mybir
    import concourse.bass as bass
    from concourse import tile

    f32 = mybir.dt.float32
    packw = nitems * W_ITEM
    nchunks = packw // CHUNK
    nwin = sum(class_counts)
    assert nchunks * CHUNK == packw

    nc = bacc.Bacc("TRN2", target_bir_lowering=False, debug=False,
                   num_devices=N_CORES)
    xt_d = nc.dram_tensor("xt", [66, 128], f32, kind="ExternalInput").ap()
    rt_d = nc.dram_tensor("rt", [6, packw], f32, kind="ExternalInput").ap()
    rc_d = nc.dram_tensor("rc", [3, packw], f32, kind="ExternalInput").ap()
    off_d = nc.dram_tensor("off", [1, nwin], mybir.dt.int32,
                           kind="ExternalInput").ap()
    out_d = nc.dram_tensor("out", [128, 3 * G], f32, kind="ExternalOutput").ap()

    AF = mybir.ActivationFunctionType
    OP = mybir.AluOpType

    with tile.TileContext(nc) as tc:
        with (
            tc.tile_pool(name="const", bufs=1) as constp,
            tc.tile_pool(name="work", bufs=2) as workp,
            tc.tile_pool(name="psum", bufs=8, space="PSUM") as psump,
        ):
            # matmul operand pairs must sit at base partitions 0/32/64,
            # matching between lhsT and rhs
            xt = constp.tile([66, 128], f32)
            nc.sync.dma_start(xt[:], xt_d[:])
            rt = constp.tile([66, packw], f32)
            nc.sync.dma_start(rt[0:2, :], rt_d[0:2, :])
            nc.sync.dma_start(rt[32:34, :], rt_d[2:4, :])
            nc.sync.dma_start(rt[64:66, :], rt_d[4:6, :])
            rc = constp.tile([65, packw], f32)
            nc.sync.dma_start(rc[0:1, :], rc_d[0:1, :])
            nc.sync.dma_start(rc[32:33, :], rc_d[1:2, :])
            nc.sync.dma_start(rc[64:65, :], rc_d[2:3, :])
            off = constp.tile([1, nwin], mybir.dt.int32)
            nc.sync.dma_start(off[:], off_d[:])

            grid = constp.tile([128, 3 * G], f32)
            grid1 = constp.tile([128, 3 * G], f32)
            nc.gpsimd.memset(grid[:], 0.0)
            nc.gpsimd.memset(grid1[:], 0.0)

            # vint: channel-interleaved packed values (c fastest)
            vint = constp.tile([128, 3 * packw], f32)
            vint3 = vint[:].rearrange("p (j c) -> p j c", c=3)

            for ch in range(nchunks):
                sl = slice(ch * CHUNK, (ch + 1) * CHUNK)
                pal = psump.tile([128, CHUNK], f32, tag="ps")
                pa = psump.tile([128, CHUNK], f32, tag="ps")
                pb = psump.tile([128, CHUNK], f32, tag="ps")
                pc0 = psump.tile([128, CHUNK], f32, tag="ps")
                pc1 = psump.tile([128, CHUNK], f32, tag="ps")
                pc2 = psump.tile([128, CHUNK], f32, tag="ps")

                nc.tensor.matmul(pal[:], xt[0:2, :], rt[0:2, sl])
                nc.tensor.matmul(pa[:], xt[32:34, :], rt[32:34, sl])
                nc.tensor.matmul(pb[:], xt[64:66, :], rt[64:66, sl])
                nc.tensor.matmul(pc0[:], xt[0:1, :], rc[0:1, sl])
                nc.tensor.matmul(pc1[:], xt[32:33, :], rc[32:33, sl])
                nc.tensor.matmul(pc2[:], xt[64:65, :], rc[64:65, sl])

                q1 = workp.tile([128, CHUNK], f32, tag="q1")
                q2 = workp.tile([128, CHUNK], f32, tag="q2")
                s1 = workp.tile([128, CHUNK], f32, tag="s1")
                s2 = workp.tile([128, CHUNK], f32, tag="s2")
                sb = workp.tile([128, CHUNK], f32, tag="sb")
                d2a = workp.tile([128, CHUNK], f32, tag="d2a")
                d2 = workp.tile([128, CHUNK], f32, tag="d2")
                dd = workp.tile([128, CHUNK], f32, tag="dd")

                # overshoot beyond segment end / before start, in 2t units
                nc.scalar.activation(q1[:], pal[:], AF.Relu)
                nc.scalar.activation(q2[:], pa[:], AF.Relu, scale=-1.0)
                nc.scalar.activation(s1[:], q1[:], AF.Square)
                nc.scalar.activation(s2[:], q2[:], AF.Square)
                nc.scalar.activation(sb[:], pb[:], AF.Square)
                nc.gpsimd.tensor_tensor(d2a[:], s1[:], s2[:], op=OP.add)
                nc.gpsimd.tensor_tensor(d2[:], d2a[:], sb[:], op=OP.add)
                nc.scalar.activation(dd[:], d2[:], AF.Sqrt)

                # w_c = (dd - 1) * col_c into channel-interleaved vint
                vch = vint3[:, sl, :]
                nc.vector.scalar_tensor_tensor(
                    vch[:, :, 0], dd[:], 1.0, pc0[:],
                    op0=OP.subtract, op1=OP.mult)
                nc.vector.scalar_tensor_tensor(
                    vch[:, :, 1], dd[:], 1.0, pc1[:],
                    op0=OP.subtract, op1=OP.mult)
                nc.vector.scalar_tensor_tensor(
                    vch[:, :, 2], dd[:], 1.0, pc2[:],
                    op0=OP.subtract, op1=OP.mult)

            # scatter: per class, min-composite window spans into the canvas
            BATCH = 8
            widx = 0
            pk = 0
            for ci in range(MAX_CLASS):
                wspan = 3 * W_ITEM * (ci + 1)
                cls_n = class_counts[ci]
                done = 0
                while done < cls_n:
                    cnt = min(BATCH, cls_n - done)
                    _, vals = nc.values_load_multi_w_load_instructions(
                        off[0:1, widx:widx + cnt],
                        engines=[nc.vector.engine],
                        min_val=0,
                        max_val=3 * (G - W_ITEM * (ci + 1)),
                        skip_runtime_bounds_check=True,
                    )
                    for val in vals:
                        g = grid if widx % 2 == 0 else grid1
                        dst = g[:, bass.ds(val, wspan)]
                        src = vint[:, 3 * W_ITEM * pk:
                                   3 * W_ITEM * pk + wspan]
                        nc.vector.tensor_tensor(dst, dst, src, op=OP.min)
                        pk += ci + 1
                        widx += 1
                    done += cnt
            assert pk == nitems and widx == nwin

            # merge accumulators, negate + store
            outt = constp.tile([128, 3 * G], f32)
            for piece in range(4):
                slp = slice(piece * 3 * G // 4, (piece + 1) * 3 * G // 4)
                nc.vector.tensor_tensor(grid[:, slp], grid[:, slp],
                                        grid1[:, slp], op=OP.min)
                nc.scalar.activation(outt[:, slp], grid[:, slp],
                                     AF.Copy, scale=-1.0)
                nc.sync.dma_start(out_d[:, slp], outt[:, slp])

    nc.compile()
    return nc


# ---------------------------------------------------------------- entry

def _prepare(strokes, thicknesses, colors):
    blocks_of, windows_per_core, t, col = _build_worklists(
        strokes, thicknesses, colors)
    class_counts = [0] * MAX_CLASS
    for c in range(N_CORES):
        per = [0] * MAX_CLASS
        for win in windows_per_core[c]:
            per[win[4] - 1] += 1
        for ci in range(MAX_CLASS):
            class_counts[ci] = max(class_counts[ci], per[ci])
    # pad class-1 count so total chunk-items is a multiple of ITEMS_PER_CHUNK
    total = sum(cc * (ci + 1) for ci, cc in enumerate(class_counts))
    rem = (-total) % ITEMS_PER_CHUNK
    class_counts[0] += rem
    class_counts = tuple(class_counts)
    in_maps = _build_tables(blocks_of, windows_per_core, t, col, class_counts)
    nitems = sum(cc * (ci + 1) for ci, cc in enumerate(class_counts))
    return blocks_of, in_maps, nitems, class_counts


def kernel(strokes, thicknesses, colors):
    _install_ntff_hook()
    from concourse.bass_utils import run_bass_kernel_spmd

    strokes = np.asarray(strokes)
    thicknesses = np.asarray(thicknesses)
    colors = np.asarray(colors)

    blocks_of, in_maps, nitems, class_counts = _prepare(
        strokes, thicknesses, colors)
    key = (nitems, class_counts)
    if key not in _PROG_CACHE:
        _PROG_CACHE[key] = _build_program(nitems, class_counts)
    nc = _PROG_CACHE[key]

    res = run_bass_kernel_spmd(nc, in_maps, list(range(N_CORES)))

    out = np.zeros((3, G, G), np.float32)
    for c in range(N_CORES):
        o = res.results[c]["out"].reshape(128, G, 3)     # (y, c) interleaved
        for half, b in enumerate(blocks_of[c]):
            rows = o[half * BH:(half + 1) * BH]          # (64, 1024, 3)
            out[:, BH * b:BH * (b + 1), :] = rows.transpose(2, 0, 1)
    return out


if __name__ == "__main__":
    rng = np.random.default_rng(0)
    s = rng.random((N, 2, 4), np.float32)
    th = rng.random((N, 1), np.float32)
    co = rng.random((N, 3), np.float32)
    g = kernel(s, th, co)
    print("out", g.shape, g.dtype, g.min(), g.max())


# revision 22
# speedup vs baseline: 1.2247x; 1.0020x over previous
"""Bezier stroke renderer on 8 Trainium2 NeuronCores (Bass/Tile SPMD kernel).

Reference semantics: 32 cubic-Bezier strokes, each sampled into a 16-segment
polyline, rasterized onto a 1024x1024 canvas: per pixel and segment,
darkness = clip((2t - dist_to_segment)/(2t), 0, 1), max over segments within a
stroke, then grid = max(grid, darkness * color) over strokes (3 channels).

Strategy (sharding: spatial split of the pixel grid by rows):
  - The canvas is split into 16 blocks of 64 rows; each core owns 2 blocks
    (greedy-balanced by estimated work), giving a [128 partitions x 1024 cols]
    canvas tile per core held in SBUF.
  - Only pixels within 2t+1 of a segment can be painted, so host code builds a
    worklist of (segment, block) column windows (1 or 2 chunks of 32 columns),
    packed back-to-back into a per-core "packed" axis.  All per-window
    parameters ship as per-core data tables, so the single SPMD instruction
    stream is identical on every core (window counts padded per width-class
    to the max over cores).
  - Distance math in the segment's tangent frame, pre-scaled by 1/(2t):
        dist/(2t) = sqrt(relu(a-L)^2 + relu(-a)^2 + b^2)
    with a,b affine in pixel coords -> TensorE matmuls (lhsT = [1; x_row],
    per-column coefficients from host tables), ACT does relu/square/sqrt,
    GPSIMD the adds, and DVE computes w_c = (dist/(2t) - 1) * col_c into a
    channel-interleaved packed buffer.
  - Scatter: per window, w is min-composited into a negated-grid SBUF
    accumulator at a register-loaded dynamic column offset (DVE tensor_tensor
    min), alternating between two accumulators to keep the pipe full.
  - Final: out = -min(acc0, acc1), negate on ACT, DMA to DRAM; the host
    reassembles block rows into the (3, 1024, 1024) canvas.
"""

import sys
import types
import contextlib
import ctypes

sys.path.insert(0, "/opt/trn_rl_repo")

import numpy as np

G = 1024
P = 16
N = 32
N_CORES = 8
BH = 64           # block height (rows)
NB = G // BH      # 16 blocks
BLOCKS_PER_CORE = NB // N_CORES
W_ITEM = 32       # columns per packed chunk-item
MAX_CLASS = 2     # scatter windows are 1..MAX_CLASS chunk-items wide
CHUNK = 512       # packed columns per matmul/PSUM chunk
ITEMS_PER_CHUNK = CHUNK // W_ITEM  # 16

_PROG_CACHE = {}
_HOOK_INSTALLED = False


def _install_ntff_hook():
    """Register the NTFF profile hook (mirrors trn_boot.py) so
    run_bass_kernel_spmd(trace=True) can measure HW exec time."""
    global _HOOK_INSTALLED
    if _HOOK_INSTALLED:
        return
    _HOOK_INSTALLED = True
    try:
        import antenv
        mod = types.ModuleType("antenv.axon_hooks")
        holder = [None]
        mod.set_axon_ntff_profile_hook = lambda h: holder.__setitem__(0, h)
        mod.get_axon_ntff_profile_hook = lambda: holder[0]
        sys.modules["antenv.axon_hooks"] = mod
        antenv.axon_hooks = mod

        lib = ctypes.CDLL("/opt/axon/libaxon_pjrt.so")
        if not hasattr(lib, "axon_start_nrt_profile"):
            return
        lib.axon_start_nrt_profile.argtypes = [
            ctypes.POINTER(ctypes.c_int64),
            ctypes.c_size_t,
        ]
        lib.axon_start_nrt_profile.restype = ctypes.c_int64
        lib.axon_stop_nrt_profile.argtypes = [ctypes.c_char_p]
        lib.axon_stop_nrt_profile.restype = ctypes.c_int64

        @contextlib.contextmanager
        def _hook(output_dir, device_ids):
            import jax
            jax.devices()
            if device_ids:
                ids = (ctypes.c_int64 * len(device_ids))(*device_ids)
                rc = lib.axon_start_nrt_profile(ids, len(device_ids))
            else:
                rc = lib.axon_start_nrt_profile(None, 0)
            if rc != 0:
                raise RuntimeError(f"axon_start_nrt_profile rc={rc}")
            try:
                yield
            finally:
                n = lib.axon_stop_nrt_profile(str(output_dir).encode())
                print(f"profile: {n} file(s) written to {output_dir}",
                      file=sys.stderr)

        mod.set_axon_ntff_profile_hook(_hook)
    except Exception:
        pass


# ---------------------------------------------------------------- host side

def _bezier_weights_f32(p):
    t = np.arange(p, dtype=np.float64)
    w1 = (p - t) ** 3 / p ** 3
    w2 = 3 * (p - t) ** 2 * t / p ** 3
    w3 = 3 * (p - t) * t ** 2 / p ** 3
    w4 = t ** 3 / p ** 3
    return np.stack([w1, w2, w3, w4]).astype(np.float32)  # (4, P)


def _polylines(strokes):
    """(N,2,4) f32 -> (N, P+1, 2) f32 polyline points in pixel units,
    mirroring reference.curve_to_stroke in float32."""
    W = _bezier_weights_f32(P)
    s = strokes.astype(np.float32)
    pts, derivs = s[:, :, :2], s[:, :, 2:]
    before = pts - derivs
    after = pts + derivs
    p1, p2, p3, p4 = pts[:, :-1], after[:, :-1], before[:, 1:], pts[:, 1:]
    cp = np.stack([p1, p2, p3, p4], axis=3)          # (N, 1, 2, 4)
    sp = np.einsum("nsdk,kp->nspd", cp, W).astype(np.float32)  # (N,1,P,2)
    sp = sp.reshape(s.shape[0], -1, 2)
    poly = np.concatenate([sp, pts[:, -1:, :]], axis=1).astype(np.float32)
    return poly * np.float32(G)


def _band_clip(v, w, pad, x0, x1):
    """Clip segment v->w (f64) to row band [x0-pad, x1+pad]; return padded,
    canvas-clamped column range [c0, c1] or None."""
    lo_x, hi_x = x0 - pad, x1 + pad
    dx = w[0] - v[0]
    if abs(dx) < 1e-12:
        if v[0] < lo_x or v[0] > hi_x:
            return None
        s0, s1 = 0.0, 1.0
    else:
        sa = (lo_x - v[0]) / dx
        sb = (hi_x - v[0]) / dx
        s0 = max(0.0, min(sa, sb))
        s1 = min(1.0, max(sa, sb))
        if s0 > s1:
            return None
    ya = v[1] + s0 * (w[1] - v[1])
    yb = v[1] + s1 * (w[1] - v[1])
    c0 = max(0.0, min(ya, yb) - pad)
    c1 = min(G - 1.0, max(ya, yb) + pad)
    if c1 < c0:
        return None
    return int(np.floor(c0)), int(np.ceil(c1))


def _build_worklists(strokes, thicknesses, colors):
    """Returns (blocks_of_core, items_per_core, t, col) where each
    items_per_core[c] is a list of (n, v(2,), w(2,), c0)."""
    poly = _polylines(strokes).astype(np.float64)          # (N, P+1, 2)
    t = np.maximum(thicknesses.astype(np.float32) * np.float32(2.0)
                   + np.float32(0.5), np.float32(0.5))[:, 0]  # f32 (N,)
    col = np.clip(colors.astype(np.float32), 0.0, 1.0)     # (N, 3)
    r = 2.0 * t.astype(np.float64)
    pad = r + 1.0

    items_by_block = [[] for _ in range(NB)]
    cost = np.zeros(NB)
    for n in range(N):
        for i in range(P):
            v = poly[n, i]
            w = poly[n, i + 1]
            for b in range(NB):
                clip = _band_clip(v, w, pad[n], BH * b, BH * b + BH - 1)
                if clip is None:
                    continue
                c0, c1 = clip
                # windows of at most MAX_CLASS chunks
                width = c1 - c0 + 1
                cstart = c0
                while width > 0:
                    nch = min(MAX_CLASS, int(np.ceil(width / W_ITEM)))
                    cc = max(0, min(cstart, G - W_ITEM * nch))
                    items_by_block[b].append((n, v, w, cc, nch))
                    cstart += W_ITEM * nch
                    width -= W_ITEM * nch
                    cost[b] += nch

    order = np.argsort(-cost)
    loads = np.zeros(N_CORES)
    blocks_of = [[] for _ in range(N_CORES)]
    for b in order:
        cands = [c for c in range(N_CORES) if len(blocks_of[c]) < BLOCKS_PER_CORE]
        c = min(cands, key=lambda c: loads[c])
        blocks_of[c].append(int(b))
        loads[c] += cost[b]
    for c in range(N_CORES):
        blocks_of[c].sort()

    items_per_core = [
        [it for b in blocks_of[c] for it in items_by_block[b]]
        for c in range(N_CORES)
    ]
    return blocks_of, items_per_core, t, col


def _build_tables(blocks_of, windows_per_core, t, col, class_counts):
    """Build per-core input tables. Windows are (n, v, w, c0, nch); each core's
    windows are grouped by chunk-class and padded to the shared class_counts.
    Returns (in_maps, nitems, nwin)."""
    nitems = sum(cc * (ci + 1) for ci, cc in enumerate(class_counts))
    nwin = sum(class_counts)
    in_maps = []
    for c in range(N_CORES):
        by_class = [[] for _ in range(MAX_CLASS)]
        for win in windows_per_core[c]:
            by_class[win[4] - 1].append(win)
        ordered = []
        for ci in range(MAX_CLASS):
            assert len(by_class[ci]) <= class_counts[ci]
            pads = class_counts[ci] - len(by_class[ci])
            ordered += by_class[ci]
            ordered += [None] * pads

        # expand windows into chunk-items
        vx = np.zeros(nitems); vy = np.zeros(nitems)
        wx = np.zeros(nitems); wy = np.zeros(nitems)
        cstart = np.zeros(nitems, np.int64)
        i2t = np.full(nitems, 1.0)
        cols = np.zeros((nitems, 3))
        valid = np.zeros(nitems, bool)
        offv = np.zeros(nwin, np.int64)
        j = 0
        for widx, win in enumerate(ordered):
            if win is None:
                nch = _class_of(widx, class_counts)
                j += nch
                continue
            n, v, w, c0, nch = win
            offv[widx] = 3 * c0
            for i in range(nch):
                vx[j], vy[j] = v
                wx[j], wy[j] = w
                cstart[j] = c0 + W_ITEM * i
                i2t[j] = 1.0 / (2.0 * np.float64(t[n]))
                cols[j] = col[n]
                valid[j] = True
                j += 1
        assert j == nitems

        dx = wx - vx
        dy = wy - vy
        L = np.hypot(dx, dy)
        safe = L > 1e-9
        taux = np.where(safe, dx / np.where(safe, L, 1.0), 1.0)
        tauy = np.where(safe, dy / np.where(safe, L, 1.0), 0.0)
        Leff = np.where(safe, L, 0.0)
        nux = -tauy
        nuy = taux

        av = vx * taux + vy * tauy
        bv = vx * nux + vy * nuy
        ycols = cstart[:, None] + np.arange(W_ITEM)[None, :]   # (nitems, 32)
        # shifted-center tangent coord and normal coord, in 2t units
        a1 = taux * i2t
        b1 = nux * i2t
        a2 = (ycols * tauy[:, None] - (av + Leff / 2.0)[:, None]) * i2t[:, None]
        b2 = (ycols * nuy[:, None] - bv[:, None]) * i2t[:, None]
        hh = (Leff / 2.0) * i2t

        dead = ~valid
        a1[dead] = 0.0; b1[dead] = 0.0; hh[dead] = 0.0
        a2[dead] = 0.0; b2[dead] = 0.0
        cols[dead] = 0.0

        packw = nitems * W_ITEM
        # a-centered rows and (a-centered - H)/(-a-centered - H) trick are not
        # needed: ship plain tangent rows plus shifted rows for the two relus
        a2u = a2 + hh[:, None]          # tangent coord from segment start
        rt = np.zeros((6, packw), np.float32)
        rt[0] = (a2u - 2.0 * hh[:, None]).ravel().astype(np.float32)  # a - L
        rt[1] = np.repeat(a1, W_ITEM).astype(np.float32)
        rt[2] = a2u.ravel().astype(np.float32)                        # a
        rt[3] = rt[1]
        rt[4] = b2.ravel().astype(np.float32)
        rt[5] = np.repeat(b1, W_ITEM).astype(np.float32)
        rc = np.stack([
            np.repeat(cols[:, 0], W_ITEM),
            np.repeat(cols[:, 1], W_ITEM),
            np.repeat(cols[:, 2], W_ITEM),
        ]).astype(np.float32)
        off = offv.astype(np.int32).reshape(1, nwin)

        xs = np.zeros(128, np.float32)
        for half, b in enumerate(blocks_of[c]):
            xs[half * BH:(half + 1) * BH] = BH * b + np.arange(BH)
        xt = np.zeros((66, 128), np.float32)
        for base in (0, 32, 64):
            xt[base] = 1.0
            xt[base + 1] = xs

        in_maps.append({"xt": xt, "rt": rt, "rc": rc, "off": off})
    return in_maps


def _class_of(widx, class_counts):
    for ci, cc in enumerate(class_counts):
        if widx < cc:
            return ci + 1
        widx -= cc
    raise IndexError


# ---------------------------------------------------------------- bass side

def _build_program(nitems, class_counts):
    import concourse.bacc as bacc
    import concourse.mybir as mybir
    import concourse.bass as bass
    from concourse import tile

    f32 = mybir.dt.float32
    packw = nitems * W_ITEM
    nchunks = packw // CHUNK
    nwin = sum(class_counts)
    assert nchunks * CHUNK == packw

    nc = bacc.Bacc("TRN2", target_bir_lowering=False, debug=False,
                   num_devices=N_CORES)
    xt_d = nc.dram_tensor("xt", [66, 128], f32, kind="ExternalInput").ap()
    rt_d = nc.dram_tensor("rt", [6, packw], f32, kind="ExternalInput").ap()
    rc_d = nc.dram_tensor("rc", [3, packw], f32, kind="ExternalInput").ap()
    off_d = nc.dram_tensor("off", [1, nwin], mybir.dt.int32,
                           kind="ExternalInput").ap()
    out_d = nc.dram_tensor("out", [128, 3 * G], f32, kind="ExternalOutput").ap()

    AF = mybir.ActivationFunctionType
    OP = mybir.AluOpType

    with tile.TileContext(nc) as tc:
        with (
            tc.tile_pool(name="const", bufs=1) as constp,
            tc.tile_pool(name="work", bufs=2) as workp,
            tc.tile_pool(name="psum", bufs=8, space="PSUM") as psump,
        ):
            # matmul operand pairs must sit at base partitions 0/32/64,
            # matching between lhsT and rhs
            xt = constp.tile([66, 128], f32)
            nc.sync.dma_start(xt[:], xt_d[:])
            rt = constp.tile([66, packw], f32)
            nc.sync.dma_start(rt[0:2, :], rt_d[0:2, :])
            nc.sync.dma_start(rt[32:34, :], rt_d[2:4, :])
            nc.sync.dma_start(rt[64:66, :], rt_d[4:6, :])
            rc = constp.tile([65, packw], f32)
            nc.sync.dma_start(rc[0:1, :], rc_d[0:1, :])
            nc.sync.dma_start(rc[32:33, :], rc_d[1:2, :])
            nc.sync.dma_start(rc[64:65, :], rc_d[2:3, :])
            off = constp.tile([1, nwin], mybir.dt.int32)
            nc.sync.dma_start(off[:], off_d[:])

            grid = constp.tile([128, 3 * G], f32)
            grid1 = constp.tile([128, 3 * G], f32)
            nc.gpsimd.memset(grid[:], 0.0)
            nc.gpsimd.memset(grid1[:], 0.0)

            # vint: channel-interleaved packed values (c fastest)
            vint = constp.tile([128, 3 * packw], f32)
            vint3 = vint[:].rearrange("p (j c) -> p j c", c=3)

            for ch in range(nchunks):
                sl = slice(ch * CHUNK, (ch + 1) * CHUNK)
                pal = psump.tile([128, CHUNK], f32, tag="ps")
                pa = psump.tile([128, CHUNK], f32, tag="ps")
                pb = psump.tile([128, CHUNK], f32, tag="ps")
                pc0 = psump.tile([128, CHUNK], f32, tag="ps")
                pc1 = psump.tile([128, CHUNK], f32, tag="ps")
                pc2 = psump.tile([128, CHUNK], f32, tag="ps")

                nc.tensor.matmul(pal[:], xt[0:2, :], rt[0:2, sl])
                nc.tensor.matmul(pa[:], xt[32:34, :], rt[32:34, sl])
                nc.tensor.matmul(pb[:], xt[64:66, :], rt[64:66, sl])
                nc.tensor.matmul(pc0[:], xt[0:1, :], rc[0:1, sl])
                nc.tensor.matmul(pc1[:], xt[32:33, :], rc[32:33, sl])
                nc.tensor.matmul(pc2[:], xt[64:65, :], rc[64:65, sl])

                q1 = workp.tile([128, CHUNK], f32, tag="q1")
                q2 = workp.tile([128, CHUNK], f32, tag="q2")
                s1 = workp.tile([128, CHUNK], f32, tag="s1")
                s2 = workp.tile([128, CHUNK], f32, tag="s2")
                sb = workp.tile([128, CHUNK], f32, tag="sb")
                d2a = workp.tile([128, CHUNK], f32, tag="d2a")
                d2 = workp.tile([128, CHUNK], f32, tag="d2")
                dd = workp.tile([128, CHUNK], f32, tag="dd")

                # overshoot beyond segment end / before start, in 2t units
                nc.scalar.activation(q1[:], pal[:], AF.Relu)
                nc.scalar.activation(q2[:], pa[:], AF.Relu, scale=-1.0)
                nc.scalar.activation(s1[:], q1[:], AF.Square)
                nc.scalar.activation(s2[:], q2[:], AF.Square)
                nc.scalar.activation(sb[:], pb[:], AF.Square)
                nc.gpsimd.tensor_tensor(d2a[:], s1[:], s2[:], op=OP.add)
                nc.gpsimd.tensor_tensor(d2[:], d2a[:], sb[:], op=OP.add)
                nc.scalar.activation(dd[:], d2[:], AF.Sqrt)

                # w_c = (dd - 1) * col_c into channel-interleaved vint
                vch = vint3[:, sl, :]
                nc.vector.scalar_tensor_tensor(
                    vch[:, :, 0], dd[:], 1.0, pc0[:],
                    op0=OP.subtract, op1=OP.mult)
                nc.vector.scalar_tensor_tensor(
                    vch[:, :, 1], dd[:], 1.0, pc1[:],
                    op0=OP.subtract, op1=OP.mult)
                nc.vector.scalar_tensor_tensor(
                    vch[:, :, 2], dd[:], 1.0, pc2[:],
                    op0=OP.subtract, op1=OP.mult)

            # scatter: per class, min-composite window spans into the canvas
            BATCH = 8
            widx = 0
            pk = 0
            for ci in range(MAX_CLASS):
                wspan = 3 * W_ITEM * (ci + 1)
                cls_n = class_counts[ci]
                done = 0
                while done < cls_n:
                    cnt = min(BATCH, cls_n - done)
                    _, vals = nc.values_load_multi_w_load_instructions(
                        off[0:1, widx:widx + cnt],
                        engines=[nc.vector.engine],
                        min_val=0,
                        max_val=3 * (G - W_ITEM * (ci + 1)),
                        skip_runtime_bounds_check=True,
                    )
                    for val in vals:
                        g = grid if widx % 2 == 0 else grid1
                        dst = g[:, bass.ds(val, wspan)]
                        src = vint[:, 3 * W_ITEM * pk:
                                   3 * W_ITEM * pk + wspan]
                        nc.vector.tensor_tensor(dst, dst, src, op=OP.min)
                        pk += ci + 1
                        widx += 1
                    done += cnt
            assert pk == nitems and widx == nwin

            # merge accumulators, negate + store
            outt = constp.tile([128, 3 * G], f32)
            for piece in range(4):
                slp = slice(piece * 3 * G // 4, (piece + 1) * 3 * G // 4)
                nc.vector.tensor_tensor(grid[:, slp], grid[:, slp],
                                        grid1[:, slp], op=OP.min)
                nc.scalar.activation(outt[:, slp], grid[:, slp],
                                     AF.Copy, scale=-1.0)
                nc.sync.dma_start(out_d[:, slp], outt[:, slp])

    nc.compile()
    return nc


# ---------------------------------------------------------------- entry

def _prepare(strokes, thicknesses, colors):
    blocks_of, windows_per_core, t, col = _build_worklists(
        strokes, thicknesses, colors)
    class_counts = [0] * MAX_CLASS
    for c in range(N_CORES):
        per = [0] * MAX_CLASS
        for win in windows_per_core[c]:
            per[win[4] - 1] += 1
        for ci in range(MAX_CLASS):
            class_counts[ci] = max(class_counts[ci], per[ci])
    # pad class-1 count so total chunk-items is a multiple of ITEMS_PER_CHUNK
    # (and ensure at least one full chunk even with an empty worklist)
    total = sum(cc * (ci + 1) for ci, cc in enumerate(class_counts))
    rem = (-total) % ITEMS_PER_CHUNK
    if total + rem == 0:
        rem = ITEMS_PER_CHUNK
    class_counts[0] += rem
    class_counts = tuple(class_counts)
    in_maps = _build_tables(blocks_of, windows_per_core, t, col, class_counts)
    nitems = sum(cc * (ci + 1) for ci, cc in enumerate(class_counts))
    return blocks_of, in_maps, nitems, class_counts


def kernel(strokes, thicknesses, colors):
    _install_ntff_hook()
    from concourse.bass_utils import run_bass_kernel_spmd

    strokes = np.asarray(strokes)
    thicknesses = np.asarray(thicknesses)
    colors = np.asarray(colors)

    blocks_of, in_maps, nitems, class_counts = _prepare(
        strokes, thicknesses, colors)
    key = (nitems, class_counts)
    if key not in _PROG_CACHE:
        _PROG_CACHE[key] = _build_program(nitems, class_counts)
    nc = _PROG_CACHE[key]

    res = run_bass_kernel_spmd(nc, in_maps, list(range(N_CORES)))

    out = np.zeros((3, G, G), np.float32)
    for c in range(N_CORES):
        o = res.results[c]["out"].reshape(128, G, 3)     # (y, c) interleaved
        for half, b in enumerate(blocks_of[c]):
            rows = o[half * BH:(half + 1) * BH]          # (64, 1024, 3)
            out[:, BH * b:BH * (b + 1), :] = rows.transpose(2, 0, 1)
    return out


if __name__ == "__main__":
    rng = np.random.default_rng(0)
    s = rng.random((N, 2, 4), np.float32)
    th = rng.random((N, 1), np.float32)
    co = rng.random((N, 3), np.float32)
    g = kernel(s, th, co)
    print("out", g.shape, g.dtype, g.min(), g.max())
